# revision 9
# baseline (speedup 1.0000x reference)
"""Trainium2 Bass kernel for HATAFormer multi-head attention.

Sharding: phase 1 shards (batch, head-group) across 8 cores — core c handles
batch b=c//4 and heads 4g..4g+3 (g=c%4): QKV projections for its 256-dim
slice, dense softmax attention with the gated local-window bias, the full
normalized attention matrix output, and the (unnormalized-then-scaled)
context. Phase 2 reshards over (batch, sequence-chunk) for the output
projection + residual + LayerNorm.

All matmuls run as float32r (full-rate fp32 on the PE). Softmax is computed
without max-subtraction (scores are O(1) here); normalization is folded into
the second exp pass as a per-row bias of -ln(sumexp).
"""
import sys

for _p in ("/opt/trn_rl_repo", "/opt/pypackages"):
    if _p not in sys.path:
        sys.path.insert(0, _p)

from contextlib import ExitStack

import numpy as np

import concourse.bass as bass
import concourse.tile as tile
from concourse import bacc, mybir
from concourse.bass_utils import run_bass_kernel_spmd

F32 = mybir.dt.float32
F32R = mybir.dt.float32r
AF = mybir.ActivationFunctionType
AX = mybir.AxisListType

B, S, D, H, WIN = 2, 2048, 1024, 16, 64
DK = D // H          # 64
EPS = 1e-5
NCORES = 8
HPC = 4              # heads per core
DOUT = HPC * DK      # 256 per-core projection slice
NKT = S // 128       # 16 k/q tiles of 128


def _base(t):
    """column base of the 256-wide band for row-tile t."""
    return min(max(t * 128 - 64, 0), S - 256)


def _jidx(t):
    return 0 if t == 0 else (2 if t == NKT - 1 else 1)


def _r(ap):
    return ap if ap.dtype == F32R else ap.bitcast(F32R)


# --------------------------------------------------------------------------
# phase 1 module
# --------------------------------------------------------------------------

def _build_phase1():
    nc = bacc.Bacc("TRN2", target_bir_lowering=False, debug=False,
                   num_devices=NCORES)
    din = {}
    for name, shape in [
        ("xqT", [D, S]), ("xkT", [D, S]), ("xvT", [D, S]),
        ("wq", [D, DOUT]), ("wk", [D, DOUT]), ("wv", [D, DOUT]),
        ("bq", [128, DOUT]), ("bk", [128, DOUT]), ("bv", [128, DOUT]),
        ("lb", [1, HPC]), ("mask3", [3, 128, 256]),
        ("ones", [1, 128]), ("ident", [128, 128]),
        ("vpones", [128, 16 * 260]),
    ]:
        din[name] = nc.dram_tensor(name, shape, F32, kind="ExternalInput").ap()
    attn_out = nc.dram_tensor("attn_out", [HPC, S, S], F32,
                              kind="ExternalOutput").ap()
    ctx_out = nc.dram_tensor("ctx_out", [S, DOUT], F32,
                             kind="ExternalOutput").ap()
    qp_out = nc.dram_tensor("qp_out", [S, DOUT], F32,
                            kind="ExternalOutput").ap()

    with tile.TileContext(nc) as tc, ExitStack() as st:
        cpool = st.enter_context(tc.tile_pool(name="consts", bufs=1))
        WQ = cpool.tile([128, 8 * DOUT], F32R, tag="wq")
        WK = cpool.tile([128, 8 * DOUT], F32R, tag="wk")
        WV = cpool.tile([128, 8 * DOUT], F32R, tag="wv")
        BQ = cpool.tile([128, DOUT], F32, tag="bq")
        BK = cpool.tile([128, DOUT], F32, tag="bk")
        BV = cpool.tile([128, DOUT], F32, tag="bv")
        MASK3 = cpool.tile([128, 3 * 256], F32, tag="mask3")
        ONES = cpool.tile([1, 128], F32, tag="ones")
        IDENT = cpool.tile([128, 128], F32, tag="ident")
        LB = cpool.tile([1, HPC], F32, tag="lb")
        SIG = cpool.tile([1, HPC], F32, tag="sig")
        for c in range(8):
            nc.sync.dma_start(out=WQ[:, c * DOUT:(c + 1) * DOUT],
                              in_=_r(din["wq"][c * 128:(c + 1) * 128, :]))
            nc.sync.dma_start(out=WK[:, c * DOUT:(c + 1) * DOUT],
                              in_=_r(din["wk"][c * 128:(c + 1) * 128, :]))
            nc.sync.dma_start(out=WV[:, c * DOUT:(c + 1) * DOUT],
                              in_=_r(din["wv"][c * 128:(c + 1) * 128, :]))
        nc.sync.dma_start(out=BQ[:], in_=din["bq"][:])
        nc.sync.dma_start(out=BK[:], in_=din["bk"][:])
        nc.sync.dma_start(out=BV[:], in_=din["bv"][:])
        for j in range(3):
            nc.sync.dma_start(out=MASK3[:, j * 256:(j + 1) * 256],
                              in_=din["mask3"][j])
        nc.sync.dma_start(out=ONES[:], in_=din["ones"][:])
        nc.sync.dma_start(out=IDENT[:], in_=din["ident"][:])
        nc.sync.dma_start(out=LB[:], in_=din["lb"][:])
        nc.scalar.activation(SIG[:], LB[:], AF.Sigmoid)

        big = st.enter_context(tc.tile_pool(name="big", bufs=1))
        QPT = [big.tile([128, S], F32R, tag=f"qpt{p}", name=f"qpt{p}")
               for p in range(2)]
        KPT = [big.tile([128, S], F32R, tag=f"kpt{p}", name=f"kpt{p}")
               for p in range(2)]
        VP = big.tile([128, NKT * (DOUT + HPC)], F32R, tag="vp")  # 65-col blks
        VBLK = DOUT + HPC  # 260
        nc.sync.dma_start(out=VP[:], in_=_r(din["vpones"][:]))

        MSIG = [big.tile([128, 3 * 256], F32, tag=f"ms{h}", name=f"ms{h}")
                for h in range(HPC)]
        SIGB = [big.tile([128, 1], F32, tag=f"sg{h}", name=f"sg{h}")
                for h in range(HPC)]

        # ---- projections (+ per-head sigmoid broadcast) ----
        with tc.tile_pool(name="ppsum", bufs=1, space="PSUM") as pp, \
             tc.tile_pool(name="tpsum", bufs=2, space="PSUM") as tp, \
             tc.tile_pool(name="xin", bufs=3) as xp, \
             tc.tile_pool(name="sd", bufs=3) as sdp:

            for h in range(HPC):
                sps = tp.tile([128, 1], F32, tag="pt")
                nc.tensor.matmul(sps[:], ONES[:], SIG[:, h:h + 1],
                                 start=True, stop=True)
                nc.vector.tensor_copy(SIGB[h][:], sps[:])
                nc.vector.tensor_scalar_mul(MSIG[h][:], MASK3[:], SIGB[h][:])

            for xin, w, bias, which in [
                (din["xqT"], WQ, BQ, "q"), (din["xkT"], WK, BK, "k"),
                (din["xvT"], WV, BV, "v"),
            ]:
                for sq in range(4):          # s-quarters of 512
                    ps = [pp.tile([128, DOUT], F32, tag=f"p{i}", name=f"p{i}")
                          for i in range(4)]
                    for dc in range(8):      # din chunks of 128
                        xt = xp.tile([128, 512], F32R, tag="xt")
                        nc.sync.dma_start(
                            out=xt[:],
                            in_=_r(xin[dc * 128:(dc + 1) * 128,
                                       sq * 512:(sq + 1) * 512]))
                        for i in range(4):
                            nc.tensor.matmul(
                                ps[i][:],
                                _r(xt[:, i * 128:(i + 1) * 128]),
                                _r(w[:, dc * DOUT:(dc + 1) * DOUT]),
                                start=(dc == 0), stop=(dc == 7))
                    for i in range(4):
                        stile = sq * 4 + i   # global s-tile 0..15
                        if which == "v":
                            for hl in range(HPC):
                                nc.vector.tensor_add(
                                    VP[:, stile * VBLK + hl * (DK + 1):
                                       stile * VBLK + hl * (DK + 1) + DK],
                                    ps[i][:, hl * DK:(hl + 1) * DK],
                                    bias[:, hl * DK:(hl + 1) * DK])
                        else:
                            sd = sdp.tile([128, DOUT], F32, tag=f"{which}sd")
                            nc.vector.tensor_add(sd[:], ps[i][:], bias[:])
                            if which == "q":
                                nc.sync.dma_start(
                                    out=qp_out[stile * 128:(stile + 1) * 128, :],
                                    in_=sd[:])
                            dst = QPT if which == "q" else KPT
                            for p in range(2):
                                tt = tp.tile([128, 128], F32, tag="pt")
                                nc.tensor.transpose(
                                    tt[:], sd[:, p * 128:(p + 1) * 128],
                                    IDENT[:])
                                nc.vector.tensor_copy(
                                    dst[p][:, stile * 128:(stile + 1) * 128],
                                    tt[:])

        # ---- attention ----
        with tc.tile_pool(name="spsum", bufs=2, space="PSUM") as sp, \
             tc.tile_pool(name="cpsum", bufs=2, space="PSUM") as cp, \
             tc.tile_pool(name="bpsum", bufs=2, space="PSUM") as bp, \
             tc.tile_pool(name="trps", bufs=2, space="PSUM") as trp, \
             tc.tile_pool(name="epool", bufs=4) as ep, \
             tc.tile_pool(name="cs", bufs=2) as csp, \
             tc.tile_pool(name="small", bufs=4) as smp, \
             tc.tile_pool(name="nl", bufs=2) as nlp, \
             tc.tile_pool(name="astg", bufs=2) as ap_, \
             tc.tile_pool(name="ctxo", bufs=4) as cop:

            for h in range(HPC):
                p, off = h // 2, 64 * (h % 2)
                NL = [nlp.tile([128, 1], F32, tag=f"nl{i}", name=f"nl{h}_{i}")
                      for i in range(NKT)]

                # section A: [k, q] layout — exp, ctx accumulation, sumexp
                for qc in range(4):
                    q0 = qc * 512
                    C = cp.tile([65, 512], F32, tag="c")
                    for kt in range(NKT):
                        ST = sp.tile([128, 512], F32, tag="st")
                        nc.tensor.matmul(
                            ST[:],
                            _r(KPT[p][off:off + 64, kt * 128:(kt + 1) * 128]),
                            _r(QPT[p][off:off + 64, q0:q0 + 512]),
                            start=True, stop=True)
                        bse = _base(kt)
                        a, bnd = max(bse, q0), min(bse + 256, q0 + 512)
                        if a < bnd:
                            j = _jidx(kt)
                            nc.vector.tensor_add(
                                ST[:, a - q0:bnd - q0],
                                ST[:, a - q0:bnd - q0],
                                MSIG[h][:, j * 256 + a - bse:
                                        j * 256 + bnd - bse])
                        E = ep.tile([128, 512], F32R, tag="e")
                        nc.scalar.activation(E[:], ST[:], AF.Exp)
                        nc.tensor.matmul(
                            C[:],
                            _r(VP[:, kt * VBLK + h * (DK + 1):
                                  kt * VBLK + h * (DK + 1) + 65]),
                            _r(E[:]),
                            start=(kt == 0), stop=(kt == NKT - 1))
                    CS = csp.tile([65, 512], F32, tag="cs")
                    nc.vector.tensor_copy(CS[:], C[:])
                    for j in range(4):
                        qt = qc * 4 + j
                        tpse = trp.tile([128, 64], F32, tag="tr")
                        nc.tensor.transpose(
                            tpse[:, 0:1], CS[64:65, j * 128:(j + 1) * 128],
                            IDENT[64:65, 64:65])
                        R = smp.tile([128, 1], F32, tag="r")
                        nc.vector.reciprocal(R[:], tpse[:, 0:1])
                        nc.scalar.activation(NL[qt][:], R[:], AF.Ln)
                        tctx = trp.tile([128, 64], F32, tag="tr")
                        nc.tensor.transpose(
                            tctx[:], CS[0:64, j * 128:(j + 1) * 128],
                            IDENT[0:64, 0:64])
                        CO = cop.tile([128, 64], F32, tag="co")
                        nc.vector.tensor_scalar_mul(CO[:], tctx[:], R[:])
                        nc.sync.dma_start(
                            out=ctx_out[qt * 128:(qt + 1) * 128,
                                        h * DK:(h + 1) * DK],
                            in_=CO[:])

                # section B: [q, k] layout — normalized attn rows out
                for qt in range(NKT):
                    ATG = ap_.tile([128, S], F32, tag="astg")
                    bse = _base(qt)
                    j = _jidx(qt)
                    for kq in range(4):
                        k0 = kq * 512
                        SB = bp.tile([128, 512], F32, tag="sb")
                        nc.tensor.matmul(
                            SB[:],
                            _r(QPT[p][off:off + 64, qt * 128:(qt + 1) * 128]),
                            _r(KPT[p][off:off + 64, k0:k0 + 512]),
                            start=True, stop=True)
                        a, bnd = max(bse, k0), min(bse + 256, k0 + 512)
                        if a < bnd:
                            nc.vector.tensor_add(
                                SB[:, a - k0:bnd - k0],
                                SB[:, a - k0:bnd - k0],
                                MSIG[h][:, j * 256 + a - bse:
                                        j * 256 + bnd - bse])
                        nc.scalar.activation(ATG[:, k0:k0 + 512], SB[:],
                                             AF.Exp, bias=NL[qt][:])
                    nc.sync.dma_start(
                        out=attn_out[h, qt * 128:(qt + 1) * 128, :],
                        in_=ATG[:])
    nc.compile()
    return nc


# --------------------------------------------------------------------------
# phase 2 module
# --------------------------------------------------------------------------

def _build_phase2():
    SC = S // 4  # 512 rows per core
    nc = bacc.Bacc("TRN2", target_bir_lowering=False, debug=False,
                   num_devices=NCORES)
    din = {}
    for name, shape in [
        ("ctxT", [D, SC]), ("qpc", [SC, D]), ("woT", [D, D]),
        ("bo", [128, D]), ("gam", [128, D]), ("bet", [128, D]),
    ]:
        din[name] = nc.dram_tensor(name, shape, F32, kind="ExternalInput").ap()
    y_out = nc.dram_tensor("y_out", [SC, D], F32, kind="ExternalOutput").ap()

    with tile.TileContext(nc) as tc, ExitStack() as st:
        cpool = st.enter_context(tc.tile_pool(name="consts", bufs=1))
        WO = cpool.tile([128, 8 * D], F32R, tag="wo")
        CX = cpool.tile([128, 8 * SC], F32R, tag="cx")
        BO = cpool.tile([128, D], F32, tag="bo")
        GAM = cpool.tile([128, D], F32, tag="gam")
        BET = cpool.tile([128, D], F32, tag="bet")
        for dc in range(8):
            nc.sync.dma_start(out=WO[:, dc * D:(dc + 1) * D],
                              in_=_r(din["woT"][dc * 128:(dc + 1) * 128, :]))
            nc.sync.dma_start(out=CX[:, dc * SC:(dc + 1) * SC],
                              in_=_r(din["ctxT"][dc * 128:(dc + 1) * 128, :]))
        nc.sync.dma_start(out=BO[:], in_=din["bo"][:])
        nc.sync.dma_start(out=GAM[:], in_=din["gam"][:])
        nc.sync.dma_start(out=BET[:], in_=din["bet"][:])

        EPS_T = cpool.tile([128, 1], F32, tag="eps")
        nc.vector.memset(EPS_T[:], EPS)

        with tc.tile_pool(name="opsum", bufs=2, space="PSUM") as op, \
             tc.tile_pool(name="work", bufs=2) as wp, \
             tc.tile_pool(name="sml", bufs=4) as smp:
            for stile in range(4):
                X = wp.tile([128, D], F32, tag="x")
                QP = wp.tile([128, D], F32, tag="qp")
                nc.sync.dma_start(
                    out=QP[:],
                    in_=din["qpc"][stile * 128:(stile + 1) * 128, :])
                for half in range(2):
                    O = op.tile([128, 512], F32, tag=f"o{half}")
                    for dc in range(8):
                        nc.tensor.matmul(
                            O[:],
                            _r(CX[:, dc * SC + stile * 128:
                                  dc * SC + (stile + 1) * 128]),
                            _r(WO[:, dc * D + half * 512:
                                  dc * D + half * 512 + 512]),
                            start=(dc == 0), stop=(dc == 7))
                    nc.vector.tensor_add(X[:, half * 512:(half + 1) * 512],
                                         O[:], BO[:, half * 512:(half + 1) * 512])
                nc.vector.tensor_add(X[:], X[:], QP[:])
                SUM = smp.tile([128, 1], F32, tag="sum")
                nc.vector.reduce_sum(SUM[:], X[:], axis=AX.X)
                MU = smp.tile([128, 1], F32, tag="mu")
                nc.vector.tensor_scalar_mul(MU[:], SUM[:], 1.0 / D)
                XC = wp.tile([128, D], F32, tag="xc")
                nc.vector.tensor_scalar_sub(XC[:], X[:], MU[:])
                SQ = wp.tile([128, D], F32, tag="sq")
                VAR = smp.tile([128, 1], F32, tag="var")
                nc.scalar.activation(SQ[:], XC[:], AF.Square, accum_out=VAR[:])
                SD = smp.tile([128, 1], F32, tag="sd")
                nc.scalar.activation(SD[:], VAR[:], AF.Sqrt,
                                     scale=1.0 / D, bias=EPS_T[:])
                RS = smp.tile([128, 1], F32, tag="rs")
                nc.vector.reciprocal(RS[:], SD[:])
                Y1 = wp.tile([128, D], F32, tag="y1")
                nc.vector.tensor_scalar_mul(Y1[:], XC[:], RS[:])
                Y2 = wp.tile([128, D], F32, tag="y2")
                nc.vector.tensor_mul(Y2[:], Y1[:], GAM[:])
                Y3 = wp.tile([128, D], F32, tag="y3")
                nc.vector.tensor_add(Y3[:], Y2[:], BET[:])
                nc.sync.dma_start(
                    out=y_out[stile * 128:(stile + 1) * 128, :], in_=Y3[:])
    nc.compile()
    return nc


_P1 = None
_P2 = None


def _mask3():
    i = np.arange(128)[:, None]
    c = np.arange(256)[None, :]
    return np.stack([
        (np.abs(i - c) <= WIN),
        (np.abs(i + 64 - c) <= WIN),
        (np.abs(i + 128 - c) <= WIN),
    ]).astype(np.float32)


def kernel(q_in, k_in, v_in, Wq, bq, Wk, bk, Wv, bv, Wo, bo,
           local_bias, gamma, beta):
    global _P1, _P2
    q_in = np.ascontiguousarray(np.asarray(q_in, np.float32))
    k_in = np.ascontiguousarray(np.asarray(k_in, np.float32))
    v_in = np.ascontiguousarray(np.asarray(v_in, np.float32))
    Wq, Wk, Wv, Wo = (np.asarray(w, np.float32) for w in (Wq, Wk, Wv, Wo))
    bq, bk, bv, bo = (np.asarray(x, np.float32) for x in (bq, bk, bv, bo))
    local_bias = np.asarray(local_bias, np.float32)
    gamma = np.asarray(gamma, np.float32)
    beta = np.asarray(beta, np.float32)

    if _P1 is None:
        _P1 = _build_phase1()
    if _P2 is None:
        _P2 = _build_phase2()

    mask3 = _mask3()
    ones = np.ones((1, 128), np.float32)
    ident = np.eye(128, dtype=np.float32)
    vpones = np.ones((128, NKT * (DOUT + HPC)), np.float32)
    xT = {b: {} for b in range(B)}
    for b in range(B):
        xT[b]["q"] = np.ascontiguousarray(q_in[b].T)
        xT[b]["k"] = np.ascontiguousarray(k_in[b].T)
        xT[b]["v"] = np.ascontiguousarray(v_in[b].T)

    in_maps = []
    for c in range(NCORES):
        b, g = c // 4, c % 4
        sl = slice(g * DOUT, (g + 1) * DOUT)
        in_maps.append({
            "xqT": xT[b]["q"], "xkT": xT[b]["k"], "xvT": xT[b]["v"],
            "wq": np.ascontiguousarray(Wq[sl, :].T),
            "wk": np.ascontiguousarray(Wk[sl, :].T / 8.0),
            "wv": np.ascontiguousarray(Wv[sl, :].T),
            "bq": np.ascontiguousarray(
                np.broadcast_to(bq[sl], (128, DOUT))),
            "bk": np.ascontiguousarray(
                np.broadcast_to(bk[sl] / 8.0, (128, DOUT))),
            "bv": np.ascontiguousarray(
                np.broadcast_to(bv[sl], (128, DOUT))),
            "lb": np.ascontiguousarray(
                local_bias.reshape(H)[g * HPC:(g + 1) * HPC].reshape(1, HPC)),
            "mask3": mask3, "ones": ones, "ident": ident,
            "vpones": vpones,
        })
    r1 = run_bass_kernel_spmd(_P1, in_maps, list(range(NCORES))).results

    attn = np.empty((B, H, S, S), np.float32)
    ctx_full = np.empty((B, S, D), np.float32)
    qp_full = np.empty((B, S, D), np.float32)
    for c in range(NCORES):
        b, g = c // 4, c % 4
        attn[b, g * HPC:(g + 1) * HPC] = r1[c]["attn_out"]
        ctx_full[b][:, g * DOUT:(g + 1) * DOUT] = r1[c]["ctx_out"]
        qp_full[b][:, g * DOUT:(g + 1) * DOUT] = r1[c]["qp_out"]

    SC = S // 4
    woT = np.ascontiguousarray(Wo.T)
    bo_b = np.ascontiguousarray(np.broadcast_to(bo, (128, D)))
    gam_b = np.ascontiguousarray(np.broadcast_to(gamma, (128, D)))
    bet_b = np.ascontiguousarray(np.broadcast_to(beta, (128, D)))
    in_maps2 = []
    for c in range(NCORES):
        b, sc = c // 4, c % 4
        rows = slice(sc * SC, (sc + 1) * SC)
        in_maps2.append({
            "ctxT": np.ascontiguousarray(ctx_full[b][rows].T),
            "qpc": np.ascontiguousarray(qp_full[b][rows]),
            "woT": woT, "bo": bo_b, "gam": gam_b, "bet": bet_b,
        })
    r2 = run_bass_kernel_spmd(_P2, in_maps2, list(range(NCORES))).results

    y = np.empty((B, S, D), np.float32)
    for c in range(NCORES):
        b, sc = c // 4, c % 4
        y[b][sc * SC:(sc + 1) * SC] = r2[c]["y_out"]
    return (y, attn)
